# revision 1
# baseline (speedup 1.0000x reference)
"""Trainium2 Bass kernel for nn_Decoder (dense transformer decoder layer).

Problem (hardcoded): B=4, S=T=1024, D=512, H=8 heads, fp32.
  h  = MHA_self(x, causal) ; x1 = LN(h + x)
  h  = MHA_cross(x1, encod_out) ; x2 = LN(h + x1)
  ff = relu(x2 @ fc1) @ fc2 ; out = LN(ff + x2)

Sharding (8 cores = 4 batch groups x 2-core pairs):
  - Self-attention: tensor-parallel over heads (4 heads/core, full S).
    Partial head-sums are combined with ONE pair ReduceScatter that also
    splits the sequence: core 2b gets rows [0,512), core 2b+1 [512,1024).
  - Everything after (LN1, cross-attn, LN2, FFN, LN3) is sequence-parallel
    on the core's 512-row half: no further collectives.  Cross-attn K/V
    (from encod_out) are computed redundantly on both cores; they are
    precomputed into DRAM scratch so the work overlaps the ReduceScatter.

Device layout: activations feature-major ([D, s] with features on
partitions); weights consumed in native [Din, Dout] layout as matmul lhsT.
Scores computed as S^T [t, s]; softmax denominator = ones-vector matmul over
partitions; causal mask applied multiplicatively post-exp (no max
subtraction: |scores| <~ 1.5).  Matmuls in float32r (~1.5e-4 rel err, 4x
faster than fp32).

Host-side folding: wq,bq pre-scaled 1/sqrt(D); per-head out_linear folded
into the final projection W2[h] = wo[h] @ wf_h; all additive constants
folded into acc_bias = bf + sum_h (bv_h @ wo_h + bo_h) @ wf_h.
"""
import math
import numpy as np

B, S, T, D, H = 4, 1024, 1024, 512, 8
P = 128
NC = 8
DC = D // P    # 4 feature chunks
TC = T // P    # 8 time chunks
SW = 512       # per-core sequence half (and s-tile width)
MC = 2048 // P  # 16 FFN hidden chunks
EPS = 1e-5
PAIRS = [[0, 1], [2, 3], [4, 5], [6, 7]]

_CACHE = {}


def _host_prep(inputs):
    import ml_dtypes
    x = np.asarray(inputs["x"], np.float32)
    enc = np.asarray(inputs["encod_out"], np.float32)
    scale = 1.0 / math.sqrt(D)

    per_phase = {}
    for p in ("sa", "ca"):
        wq = np.asarray(inputs[p + "_wq"], np.float32) * scale
        bq = np.asarray(inputs[p + "_bq"], np.float32) * scale
        wk = np.asarray(inputs[p + "_wk"], np.float32)
        bk = np.asarray(inputs[p + "_bk"], np.float32)
        wv = np.asarray(inputs[p + "_wv"], np.float32)
        bv = np.asarray(inputs[p + "_bv"], np.float32)
        wo = np.asarray(inputs[p + "_wo"], np.float32)
        bo = np.asarray(inputs[p + "_bo"], np.float32)
        wf = np.asarray(inputs[p + "_wf"], np.float32).reshape(H, D, D)
        bf = np.asarray(inputs[p + "_bf"], np.float32)
        w2 = np.einsum("hfg,hgk->hfk", wo.astype(np.float64),
                       wf.astype(np.float64)).astype(np.float32)
        acc = bf.astype(np.float64).copy()
        for h in range(H):
            acc += (bv[h].astype(np.float64) @ wo[h].astype(np.float64)
                    + bo[h].astype(np.float64)) @ wf[h].astype(np.float64)
        per_phase[p] = dict(wq=wq, bq=bq, wk=wk, bk=bk, wv=wv, w2=w2,
                            acc=acc.astype(np.float32))

    fc1_w = np.asarray(inputs["fc1_w"], np.float32)
    fc1_b = np.asarray(inputs["fc1_b"], np.float32)
    fc2_w = np.asarray(inputs["fc2_w"], np.float32)
    fc2_b = np.asarray(inputs["fc2_b"], np.float32)
    lns = {f"ln{i}_{k}": np.asarray(inputs[f"ln{i}_{k}"], np.float32)
           for i in (1, 2, 3) for k in ("g", "b")}

    # causal masks for the diagonal [128, 512] blocks: keep where 128*r + p <= c
    pp = np.arange(P)[:, None]
    cc = np.arange(SW)[None, :]
    masks = np.stack([(128 * r + pp <= cc) for r in range(4)], axis=1)
    masks = masks.astype(ml_dtypes.bfloat16)
    ones = np.ones((P,), np.float32)
    ones_bf = np.ones((P,), ml_dtypes.bfloat16)

    in_maps = []
    for c in range(NC):
        b, half = c // 2, c % 2
        hs = slice(4 * half, 4 * half + 4)
        ssl = slice(half * SW, (half + 1) * SW)
        m = {
            "xt": np.ascontiguousarray(x[b].T),
            "et": np.ascontiguousarray(enc[b].T).astype(ml_dtypes.bfloat16),
            "x_res1": np.ascontiguousarray(x[b].T[:, ssl]),
            "masks": masks,
            "ones": ones,
            "ones_bf": ones_bf,
            "fc1_w": fc1_w, "fc1_b": fc1_b, "fc2_w": fc2_w,
            "ffn_bias": fc2_b,
        }
        pp_ = per_phase["sa"]
        m["sa_wq"] = np.ascontiguousarray(pp_["wq"][hs])
        m["sa_bq"] = np.ascontiguousarray(pp_["bq"][hs])
        m["sa_wk"] = np.ascontiguousarray(pp_["wk"][hs])
        m["sa_bk"] = np.ascontiguousarray(pp_["bk"][hs])
        m["sa_wv"] = np.ascontiguousarray(pp_["wv"][hs])
        m["sa_w2"] = np.ascontiguousarray(pp_["w2"][hs])
        m["sa_acc_half"] = pp_["acc"] / 2.0
        pp_ = per_phase["ca"]
        for k in ("wq", "bq", "bk", "w2"):
            m["ca_" + k] = pp_[k]
        m["ca_wk"] = pp_["wk"].astype(ml_dtypes.bfloat16)
        m["ca_wv"] = pp_["wv"].astype(ml_dtypes.bfloat16)
        m["ca_acc"] = pp_["acc"]
        for k, v in lns.items():
            m[k] = v
        in_maps.append(m)
    return in_maps


def build_program():
    import concourse.bacc as bacc
    import concourse.mybir as mybir
    import concourse.tile as tile

    F32 = mybir.dt.float32
    F32R = mybir.dt.float32r
    BF16 = mybir.dt.bfloat16
    AF = mybir.ActivationFunctionType
    OP = mybir.AluOpType

    nc = bacc.Bacc(None, target_bir_lowering=False, num_devices=NC)

    # ---- DRAM I/O ----
    xt_d = nc.dram_tensor("xt", [D, S], F32, kind="ExternalInput")
    et_d = nc.dram_tensor("et", [D, T], BF16, kind="ExternalInput")
    xres1_d = nc.dram_tensor("x_res1", [D, SW], F32, kind="ExternalInput")
    masks_d = nc.dram_tensor("masks", [P, 4, SW], BF16, kind="ExternalInput")
    ones_d = nc.dram_tensor("ones", [P], F32, kind="ExternalInput")
    onesbf_d = nc.dram_tensor("ones_bf", [P], BF16, kind="ExternalInput")
    sa_d = dict(
        wq=nc.dram_tensor("sa_wq", [4, D, D], F32, kind="ExternalInput"),
        bq=nc.dram_tensor("sa_bq", [4, D], F32, kind="ExternalInput"),
        wk=nc.dram_tensor("sa_wk", [4, D, D], F32, kind="ExternalInput"),
        bk=nc.dram_tensor("sa_bk", [4, D], F32, kind="ExternalInput"),
        wv=nc.dram_tensor("sa_wv", [4, D, D], F32, kind="ExternalInput"),
        w2=nc.dram_tensor("sa_w2", [4, D, D], F32, kind="ExternalInput"),
        acc=nc.dram_tensor("sa_acc_half", [D], F32, kind="ExternalInput"),
    )
    ca_d = dict(
        wq=nc.dram_tensor("ca_wq", [H, D, D], F32, kind="ExternalInput"),
        bq=nc.dram_tensor("ca_bq", [H, D], F32, kind="ExternalInput"),
        wk=nc.dram_tensor("ca_wk", [H, D, D], BF16, kind="ExternalInput"),
        bk=nc.dram_tensor("ca_bk", [H, D], F32, kind="ExternalInput"),
        wv=nc.dram_tensor("ca_wv", [H, D, D], BF16, kind="ExternalInput"),
        w2=nc.dram_tensor("ca_w2", [H, D, D], F32, kind="ExternalInput"),
        acc=nc.dram_tensor("ca_acc", [D], F32, kind="ExternalInput"),
    )
    fc1w_d = nc.dram_tensor("fc1_w", [D, 2048], F32, kind="ExternalInput")
    fc1b_d = nc.dram_tensor("fc1_b", [2048], F32, kind="ExternalInput")
    fc2w_d = nc.dram_tensor("fc2_w", [2048, D], F32, kind="ExternalInput")
    ffnb_d = nc.dram_tensor("ffn_bias", [D], F32, kind="ExternalInput")
    ln_d = {f"ln{i}_{k}": nc.dram_tensor(f"ln{i}_{k}", [D], F32, kind="ExternalInput")
            for i in (1, 2, 3) for k in ("g", "b")}
    outt_d = nc.dram_tensor("outt", [D, SW], F32, kind="ExternalOutput")

    r32 = lambda ap: ap.bitcast(F32R)

    with tile.TileContext(nc, pool_alloc_mode="queue") as tc:
        with tc.tile_pool(name="const", bufs=1) as constp, \
             tc.tile_pool(name="resid", bufs=2) as residp, \
             tc.tile_pool(name="smalls", bufs=3) as smallp, \
             tc.tile_pool(name="stats", bufs=4) as statp, \
             tc.tile_pool(name="pp", bufs=3, space="PSUM") as pp, \
             tc.tile_pool(name="pd", bufs=1, space="PSUM") as pdp, \
             tc.tile_pool(name="po", bufs=2, space="PSUM") as pop, \
             tc.tile_pool(name="pw", bufs=2, space="PSUM") as pwp, \
             tc.tile_pool(name="ca_kvw", bufs=3) as kvwp, \
             tc.tile_pool(name="dram", bufs=1, space="DRAM") as dramp:

            # ---- constants ----
            eps_sb = constp.tile([1, 1], F32, name="eps_sb")
            nc.vector.memset(eps_sb[:], EPS)
            xt_sb = residp.tile([P, DC, S], F32R, name="xt_sb", tag="resid")
            for c in range(DC):
                nc.sync.dma_start(
                    out=xt_sb[:, c, :],
                    in_=r32(xt_d.ap().rearrange("(c p) s -> p c s", p=P)[:, c, :]))
            xres1_sb = residp.tile([P, DC, SW], F32R, name="xres1_sb", tag="xres",
                                   bufs=1)

            et_sb = residp.tile([P, DC, T], BF16, name="et_sb", tag="et", bufs=1)

            ones_col = constp.tile([P, 1], F32R, name="ones_col")
            nc.gpsimd.dma_start(out=ones_col[:],
                                in_=r32(ones_d.ap().rearrange("(p a) -> p a", a=1)))
            ones_row = constp.tile([1, P], F32R, name="ones_row")
            nc.gpsimd.dma_start(out=ones_row[:],
                                in_=r32(ones_d.ap().rearrange("(a p) -> a p", a=1)))
            ones_col_bf = constp.tile([P, 1], BF16, name="ones_col_bf")
            nc.gpsimd.dma_start(out=ones_col_bf[:],
                                in_=onesbf_d.ap().rearrange("(p a) -> p a", a=1))
            masks_sb = constp.tile([P, 4, SW], BF16, name="masks_sb")

            def vec_to_pc(dram_ap, name, nch):
                t = constp.tile([P, nch], F32, name=name)
                nc.gpsimd.dma_start(out=t[:],
                                    in_=dram_ap.rearrange("(c p) -> p c", p=P))
                return t

            bias_sb = {}
            for pn, dd, nh in (("sa", sa_d, 4), ("ca", ca_d, H)):
                for k in ("bq", "bk"):
                    t = constp.tile([P, nh, 4], F32, name=f"{pn}_{k}_sb")
                    nc.gpsimd.dma_start(
                        out=t[:], in_=dd[k].ap().rearrange("h (c p) -> p h c", p=P))
                    bias_sb[pn, k] = t
                bias_sb[pn, "acc"] = vec_to_pc(dd["acc"].ap(), f"{pn}_acc_sb", DC)
            grow_sb = constp.tile([1, 3, DC, P], F32R, name="ln_grow")
            for _i in (1, 2, 3):
                nc.gpsimd.dma_start(
                    out=grow_sb[:, _i - 1, :, :],
                    in_=r32(ln_d[f"ln{_i}_g"].ap().rearrange(
                        "(a c p) -> a c p", a=1, p=P)))
            nc.gpsimd.dma_start(out=masks_sb[:], in_=masks_d.ap())
            fc1b_sb = vec_to_pc(fc1b_d.ap(), "fc1b_sb", MC)
            ffnb_sb = vec_to_pc(ffnb_d.ap(), "ffnb_sb", DC)
            ln_sb = {k: vec_to_pc(v.ap(), k + "_sb", DC) for k, v in ln_d.items()}

            cc_in = dramp.tile([2, D, SW], F32, name="cc_in")
            cc_half = dramp.tile([D, SW], F32, name="cc_half")
            ktd = [dramp.tile([P, DC, T], BF16, name=f"ktd{h}") for h in range(H)]
            vd = [dramp.tile([P, TC, D], BF16, name=f"vd{h}") for h in range(H)]

            ca_kvw_tiles = {}
            ln1_anchor = [None]

            def load_ca_kvw(h, eng):
                wk_sb = kvwp.tile([P, DC, D], BF16, name=f"ca_wkp_{h}", tag="wk")
                eng.dma_start(out=wk_sb[:], in_=ca_d["wk"].ap()[h]
                              .rearrange("(c p) f -> p c f", p=P))
                wv_sb = kvwp.tile([P, DC, D], BF16, name=f"ca_wvp_{h}", tag="wv")
                eng.dma_start(out=wv_sb[:], in_=ca_d["wv"].ap()[h]
                              .rearrange("(c p) f -> p c f", p=P))
                return wk_sb, wv_sb

            def proj_kv(xkv_sb, wk_sb, wv_sb, kt, v_sb, pn, h):
                """K^T [f, t] and V [t, f] projections for one head."""
                for fc in range(DC):
                    for tt in range(T // SW):
                        tsl = slice(tt * SW, (tt + 1) * SW)
                        ps = pp.tile([P, SW], F32, name=f"{pn}_kp_{h}_{fc}_{tt}",
                                     tag="pp")
                        for c in range(DC):
                            nc.tensor.matmul(ps[:], wk_sb[:, c, fc * P:(fc + 1) * P],
                                             xkv_sb[:, c, tsl],
                                             start=(c == 0), stop=(c == DC - 1))
                        nc.scalar.activation(kt[:, fc, tsl], ps[:], AF.Identity,
                                             bias=bias_sb[pn, "bk"][:, h, fc:fc + 1])
                last = None
                for tci in range(TC):
                    ps = pp.tile([P, D], F32, name=f"{pn}_vp_{h}_{tci}", tag="pp")
                    for c in range(DC):
                        nc.tensor.matmul(ps[:], xkv_sb[:, c, tci * P:(tci + 1) * P],
                                         wv_sb[:, c, :],
                                         start=(c == 0), stop=(c == DC - 1))
                    last = nc.any.tensor_copy(v_sb[:, tci, :], ps[:])
                return last

            def attn_core(pn, tag, qt, kt, v_sb, w2_sb, e_sb, avp, f_sb, fdst_sl,
                          n_tc, causal_st, first_head):
                """scores -> exp(mask) -> denom -> AV -> normalize -> W2 -> F."""
                psum_d = pdp.tile([1, SW], F32, name=f"{pn}_d_{tag}", tag="pd")
                for tci in range(n_tc):
                    ps = pp.tile([P, SW], F32, name=f"{pn}_sp_{tag}_{tci}", tag="pp")
                    for fc in range(DC):
                        nc.tensor.matmul(ps[:], kt[:, fc, tci * P:(tci + 1) * P],
                                         qt[:, fc, :],
                                         start=(fc == 0), stop=(fc == DC - 1))
                    nc.scalar.activation(e_sb[:, tci, :], ps[:], AF.Exp)
                    if causal_st is not None and tci >= 4 * causal_st:
                        r = tci - 4 * causal_st
                        nc.vector.tensor_tensor(e_sb[:, tci, :], e_sb[:, tci, :],
                                                masks_sb[:, r, :], OP.mult)
                    nc.tensor.matmul(psum_d[:], ones_col_bf[:], e_sb[:, tci, :],
                                     start=(tci == 0), stop=(tci == n_tc - 1))
                recip = statp.tile([1, SW], F32R, name=f"{pn}_rc_{tag}", tag="st")
                with nc.allow_low_precision(reason="f32r feed for bcast matmul"):
                    nc.vector.reciprocal(recip[:], psum_d[:])
                psum_rb = pp.tile([P, SW], F32, name=f"{pn}_rb_{tag}", tag="pp")
                nc.tensor.matmul(psum_rb[:], ones_row[:], recip[:],
                                 start=True, stop=True)
                rb = smallp.tile([P, SW], F32, name=f"{pn}_rbs_{tag}", tag="sm")
                nc.scalar.activation(rb[:], psum_rb[:], AF.Copy)

                avns = []
                for fc in range(DC):
                    po = pop.tile([P, SW], F32, name=f"{pn}_o_{tag}_{fc}", tag="po")
                    for tci in range(n_tc):
                        nc.tensor.matmul(po[:], v_sb[:, tci, fc * P:(fc + 1) * P],
                                         e_sb[:, tci, :],
                                         start=(tci == 0), stop=(tci == n_tc - 1))
                    avn = avp.tile([P, SW], F32R, name=f"{pn}_avn_{tag}_{fc}",
                                   tag="avn")
                    nc.vector.tensor_tensor(avn[:], po[:], rb[:], OP.mult)
                    avns.append(avn)
                for gc in range(DC):
                    pw = pwp.tile([P, SW], F32, name=f"{pn}_pw_{tag}_{gc}", tag="pw")
                    for fc in range(DC):
                        nc.tensor.matmul(pw[:], w2_sb[:, fc, gc * P:(gc + 1) * P],
                                         avns[fc][:],
                                         start=(fc == 0), stop=(fc == DC - 1))
                    if first_head:
                        nc.vector.tensor_scalar_add(
                            f_sb[:, gc, fdst_sl], pw[:],
                            bias_sb[pn, "acc"][:, gc:gc + 1])
                    else:
                        nc.vector.tensor_add(f_sb[:, gc, fdst_sl],
                                             f_sb[:, gc, fdst_sl], pw[:])

            def layernorm(src, resid_sb, dst, g_sb, b_sb, gri):
                """dst = LN(src + resid) over d, on [P, DC, SW] tiles.

                src: DRAM AP [D, SW] (DMA'd in) or SBUF tile [P, DC, SW]."""
                if src.tensor.shape[0] == D:
                    nc.sync.dma_start(
                        out=dst[:], in_=r32(src.rearrange("(c p) s -> p c s", p=P)))
                    for c in range(DC):
                        nc.vector.tensor_add(dst[:, c, :], dst[:, c, :],
                                             resid_sb[:, c, :])
                else:
                    for c in range(DC):
                        nc.vector.tensor_add(dst[:, c, :], src[:, c, :],
                                             resid_sb[:, c, :])
                psum_sum = pp.tile([1, SW], F32, name="ln_sum", tag="pp")
                psum_ssq = pp.tile([1, SW], F32, name="ln_ssq", tag="pp")
                for c in range(DC):
                    sq = smallp.tile([P, SW], F32R, name=f"ln_sq_{c}", tag="sm")
                    nc.scalar.activation(sq[:], dst[:, c, :], AF.Square)
                    nc.tensor.matmul(psum_sum[:], ones_col[:], dst[:, c, :],
                                     start=(c == 0), stop=(c == DC - 1))
                    nc.tensor.matmul(psum_ssq[:], ones_col[:], sq[:],
                                     start=(c == 0), stop=(c == DC - 1))
                mean = statp.tile([1, SW], F32R, name="ln_mean", tag="st")
                nc.scalar.activation(mean[:], psum_sum[:], AF.Copy, scale=1.0 / D)
                msq = statp.tile([1, SW], F32, name="ln_msq", tag="st")
                nc.scalar.activation(msq[:], psum_ssq[:], AF.Copy, scale=1.0 / D)
                var = statp.tile([1, SW], F32, name="ln_var", tag="st")
                nc.vector.tensor_tensor(var[:], mean[:], mean[:], OP.mult)
                nc.vector.tensor_sub(var[:], msq[:], var[:])
                std = statp.tile([1, SW], F32, name="ln_std", tag="st")
                nc.scalar.activation(std[:], var[:], AF.Sqrt, bias=eps_sb[:])
                rstd = statp.tile([1, SW], F32R, name="ln_rstd", tag="st")
                with nc.allow_low_precision(reason="f32r feed for bcast matmul"):
                    nc.vector.reciprocal(rstd[:], std[:])
                mr = statp.tile([1, SW], F32R, name="ln_mr", tag="st")
                nc.vector.tensor_tensor(mr[:], mean[:], rstd[:], OP.mult)
                for c in range(DC):
                    psum_rb = pp.tile([P, SW], F32, name=f"ln_rb_{c}", tag="pp")
                    nc.tensor.matmul(psum_rb[:], grow_sb[:, gri, c, :], rstd[:],
                                     start=True, stop=True)
                    psum_mb = pp.tile([P, SW], F32, name=f"ln_mb_{c}", tag="pp")
                    nc.tensor.matmul(psum_mb[:], grow_sb[:, gri, c, :], mr[:],
                                     start=True, stop=True)
                    tmp = smallp.tile([P, SW], F32, name=f"ln_t_{c}", tag="sm")
                    nc.vector.tensor_tensor(tmp[:], dst[:, c, :], psum_rb[:],
                                            OP.mult)
                    nc.vector.scalar_tensor_tensor(
                        dst[:, c, :], tmp[:], b_sb[:, c:c + 1], psum_mb[:],
                        OP.add, OP.subtract)

            # ================ self-attention (head-split, full S) =============
            with tc.tile_pool(name="sa_w", bufs=1) as wp, \
                 tc.tile_pool(name="sa_w2p", bufs=2) as w2p, \
                 tc.tile_pool(name="sa_qkv", bufs=1) as qkvp, \
                 tc.tile_pool(name="sa_e", bufs=1) as ep, \
                 tc.tile_pool(name="sa_av", bufs=4) as avp, \
                 tc.tile_pool(name="sa_f", bufs=1) as fp:
                f_sb = fp.tile([P, DC, S], F32, name="sa_f")
                for h in range(4):
                    if h == 2:
                        nc.gpsimd.dma_start(
                            out=et_sb[:],
                            in_=et_d.ap().rearrange("(c p) s -> p c s", p=P))
                        nc.gpsimd.dma_start(
                            out=xres1_sb[:],
                            in_=r32(xres1_d.ap().rearrange("(c p) s -> p c s", p=P)))
                        for hh in range(3):
                            ca_kvw_tiles[hh] = load_ca_kvw(hh, nc.gpsimd)
                    wk_sb = wp.tile([P, DC, D], F32R, name=f"sa_wk_{h}", tag="wk")
                    nc.sync.dma_start(out=wk_sb[:], in_=r32(
                        sa_d["wk"].ap()[h].rearrange("(c p) f -> p c f", p=P)))
                    wv_sb = wp.tile([P, DC, D], F32R, name=f"sa_wv_{h}", tag="wv")
                    nc.sync.dma_start(out=wv_sb[:], in_=r32(
                        sa_d["wv"].ap()[h].rearrange("(c p) f -> p c f", p=P)))
                    wq_sb = wp.tile([P, DC, D], F32R, name=f"sa_wq_{h}", tag="wq")
                    nc.sync.dma_start(out=wq_sb[:], in_=r32(
                        sa_d["wq"].ap()[h].rearrange("(c p) f -> p c f", p=P)))
                    w2_sb = w2p.tile([P, DC, D], F32R, name=f"sa_w2_{h}", tag="w2")
                    nc.sync.dma_start(out=w2_sb[:], in_=r32(
                        sa_d["w2"].ap()[h].rearrange("(c p) f -> p c f", p=P)))

                    kt = qkvp.tile([P, DC, T], F32R, name=f"sa_kt_{h}", tag="kt")
                    v_sb = qkvp.tile([P, TC, D], BF16, name=f"sa_v_{h}", tag="v")
                    proj_kv(xt_sb, wk_sb, wv_sb, kt, v_sb, "sa", h)

                    for st in range(2):
                        ssl = slice(st * SW, (st + 1) * SW)
                        n_tc = 4 * (st + 1)
                        qt = qkvp.tile([P, DC, SW], F32R, name=f"sa_qt_{h}_{st}",
                                       tag="qt")
                        for fc in range(DC):
                            ps = pp.tile([P, SW], F32, name=f"sa_qp_{h}_{st}_{fc}",
                                         tag="pp")
                            for c in range(DC):
                                nc.tensor.matmul(
                                    ps[:], wq_sb[:, c, fc * P:(fc + 1) * P],
                                    xt_sb[:, c, ssl],
                                    start=(c == 0), stop=(c == DC - 1))
                            nc.vector.tensor_scalar_add(
                                qt[:, fc, :], ps[:],
                                bias_sb["sa", "bq"][:, h, fc:fc + 1])
                        e_sb = ep.tile([P, TC, SW], BF16, name=f"sa_e_{h}_{st}",
                                       tag="e")
                        attn_core("sa", f"{h}_{st}", qt, kt, v_sb, w2_sb, e_sb,
                                  avp, f_sb, ssl, n_tc, st, h == 0)
                for half in range(2):
                    nc.gpsimd.dma_start(
                        out=cc_in[half].rearrange("(c p) s -> p c s", p=P),
                        in_=f_sb[:, :, half * SW:(half + 1) * SW])

            # one pair collective: reduce partial head-sums + scatter seq halves
            nc.gpsimd.collective_compute(
                "ReduceScatter", mybir.AluOpType.add, replica_groups=PAIRS,
                ins=[cc_in.opt()], outs=[cc_half.opt()])

            # ---- cross-attention K/V precompute (overlaps the collective) ----
            with tc.tile_pool(name="ca_kvp", bufs=3) as kvpp:
                for h in range(H):
                    if h >= 3:
                        ca_kvw_tiles[h] = load_ca_kvw(h, nc.sync)
                    wk_sb, wv_sb = ca_kvw_tiles[h]
                    kt = kvpp.tile([P, DC, T], BF16, name=f"ca_ktp_{h}", tag="ktp")
                    v_sb = kvpp.tile([P, TC, D], BF16, name=f"ca_vp_{h}", tag="vp")
                    _last = proj_kv(et_sb, wk_sb, wv_sb, kt, v_sb, "ca", h)
                    if h == 4:
                        ln1_anchor[0] = _last
                    nc.gpsimd.dma_start(out=ktd[h][:], in_=kt[:])
                    nc.gpsimd.dma_start(out=vd[h][:], in_=v_sb[:])

            # ---- LN1 on my sequence half ----
            # Deprioritize: its input waits on the collective; without this the
            # scheduler interleaves LN1's ACT ops ahead of the K/V-precompute
            # evictions and stalls the in-order ACT stream on the RS wait.
            x1_sb = residp.tile([P, DC, SW], F32R, name="x1_sb", tag="resid")
            from concourse.tile import add_dep_helper as _adh
            _bb = nc.main_func.blocks[-1]
            _n0 = len(_bb.instructions)
            layernorm(cc_half.opt(), xres1_sb, x1_sb, ln_sb["ln1_g"], ln_sb["ln1_b"], 0)
            if ln1_anchor[0] is not None:
                for _ins in list(_bb.instructions)[_n0:]:
                    _adh(_ins, ln1_anchor[0].ins, sync=False,
                         reason="order LN1 after precompute h4 (RS-wait inversion)")

            # ================ cross-attention (seq-split, all heads) ==========
            with tc.tile_pool(name="ca_w", bufs=1) as wp, \
                 tc.tile_pool(name="ca_w2p", bufs=2) as w2p, \
                 tc.tile_pool(name="ca_kv", bufs=3) as kvp, \
                 tc.tile_pool(name="ca_qt", bufs=1) as qtp, \
                 tc.tile_pool(name="ca_e", bufs=2) as ep, \
                 tc.tile_pool(name="ca_av", bufs=5) as avp, \
                 tc.tile_pool(name="ca_f", bufs=1) as fp:
                f2_sb = fp.tile([P, DC, SW], F32, name="ca_f")
                for h in range(H):
                    kt = kvp.tile([P, DC, T], BF16, name=f"ca_kt_{h}", tag="kt")
                    nc.sync.dma_start(out=kt[:], in_=ktd[h][:])
                    v_sb = kvp.tile([P, TC, D], BF16, name=f"ca_v_{h}", tag="v")
                    nc.sync.dma_start(out=v_sb[:], in_=vd[h][:])
                    wq_sb = wp.tile([P, DC, D], F32R, name=f"ca_wq_{h}", tag="wq")
                    nc.sync.dma_start(out=wq_sb[:], in_=r32(
                        ca_d["wq"].ap()[h].rearrange("(c p) f -> p c f", p=P)))
                    w2_sb = w2p.tile([P, DC, D], F32R, name=f"ca_w2_{h}", tag="w2")
                    nc.sync.dma_start(out=w2_sb[:], in_=r32(
                        ca_d["w2"].ap()[h].rearrange("(c p) f -> p c f", p=P)))
                    qt = qtp.tile([P, DC, SW], BF16, name=f"ca_qt_{h}", tag="qt")
                    for fc in range(DC):
                        ps = pp.tile([P, SW], F32, name=f"ca_qp_{h}_{fc}", tag="pp")
                        for c in range(DC):
                            nc.tensor.matmul(ps[:], wq_sb[:, c, fc * P:(fc + 1) * P],
                                             x1_sb[:, c, :],
                                             start=(c == 0), stop=(c == DC - 1))
                        nc.vector.tensor_scalar_add(
                            qt[:, fc, :], ps[:],
                            bias_sb["ca", "bq"][:, h, fc:fc + 1])
                    e_sb = ep.tile([P, TC, SW], BF16, name=f"ca_e_{h}", tag="e")
                    attn_core("ca", str(h), qt, kt, v_sb, w2_sb, e_sb,
                              avp, f2_sb, slice(0, SW), TC, None, h == 0)

                # ---- LN2 ----
                x2_sb = residp.tile([P, DC, SW], F32R, name="x2_sb", tag="resid")
                layernorm(f2_sb, x1_sb, x2_sb, ln_sb["ln2_g"], ln_sb["ln2_b"], 1)

            # ================ FFN (seq-split, full hidden) ====================
            with tc.tile_pool(name="ffn_w", bufs=1) as fwp, \
                 tc.tile_pool(name="ffn_h", bufs=1) as fhp:
                fc1_sb = fwp.tile([P, DC, 2048], F32R, name="fc1_sb")
                for mg in range(4):
                    nc.sync.dma_start(
                        out=fc1_sb[:, :, mg * SW:(mg + 1) * SW], in_=r32(
                            fc1w_d.ap().rearrange("(c p) m -> p c m", p=P)
                            [:, :, mg * SW:(mg + 1) * SW]))
                fc2_sb = fwp.tile([P, MC, D], F32R, name="fc2_sb")
                nc.sync.dma_start(out=fc2_sb[:], in_=r32(
                    fc2w_d.ap().rearrange("(c p) g -> p c g", p=P)))
                h_sb = fhp.tile([P, MC, SW], F32R, name="h_sb")
                f3_sb = fhp.tile([P, DC, SW], F32, name="f3_sb")
                for mc in range(MC):
                    ps = pp.tile([P, SW], F32, name=f"f1_{mc}", tag="pp")
                    for c in range(DC):
                        nc.tensor.matmul(ps[:], fc1_sb[:, c, mc * P:(mc + 1) * P],
                                         x2_sb[:, c, :],
                                         start=(c == 0), stop=(c == DC - 1))
                    nc.scalar.activation(h_sb[:, mc, :], ps[:], AF.Relu,
                                         bias=fc1b_sb[:, mc:mc + 1])
                for gc in range(DC):
                    ps = pwp.tile([P, SW], F32, name=f"f2_{gc}", tag="pw")
                    for mc in range(MC):
                        nc.tensor.matmul(ps[:], fc2_sb[:, mc, gc * P:(gc + 1) * P],
                                         h_sb[:, mc, :],
                                         start=(mc == 0), stop=(mc == MC - 1))
                    nc.vector.tensor_scalar_add(f3_sb[:, gc, :], ps[:],
                                                ffnb_sb[:, gc:gc + 1])

                # ---- LN3 + output ----
                out_sb = residp.tile([P, DC, SW], F32R, name="out_sb", tag="resid")
                layernorm(f3_sb, x2_sb, out_sb, ln_sb["ln3_g"], ln_sb["ln3_b"], 2)
                for c in range(DC):
                    nc.sync.dma_start(
                        out=outt_d.ap().rearrange("(c p) s -> p c s", p=P)[:, c, :],
                        in_=out_sb[:, c, :].bitcast(F32))

    nc.compile()
    return nc


def get_program():
    if "nc" not in _CACHE:
        _CACHE["nc"] = build_program()
    return _CACHE["nc"]


def kernel(**inputs) -> np.ndarray:
    from concourse.bass_utils import run_bass_kernel_spmd
    nc = get_program()
    in_maps = _host_prep(inputs)
    res = run_bass_kernel_spmd(nc, in_maps, core_ids=list(range(NC)))
    out = np.empty((B, S, D), np.float32)
    for b in range(B):
        out[b, 0:SW] = res.results[2 * b]["outt"].T
        out[b, SW:S] = res.results[2 * b + 1]["outt"].T
    return out

